# revision 13
# baseline (speedup 1.0000x reference)
"""Trainium2 Bass kernel for nn_MultiAgentsSummarizer (pointer-generator style
multi-agent summarizer distribution).

Math (per batch b, with T=64 target positions, A=4 agents, S=512 source tokens,
V=32000 vocab, EXT_V=33000 extended vocab):

    coef[t]   = sum_a agent_attn[t,a] * gen[t,a]
    out[t,v]  = coef[t] * vocab_probs[t,v]            (v <  V;  0 for v >= V)
    out[t, article[a,s]] += agent_attn[t,a]*(1-gen[t,a]) * agentwise_attn[t,a,s]

Strategy: one batch element per NeuronCore (B=8 = n_cores). Device work runs in
a v-major staging layout S[row, t] (fp16, x4096 scaling for fp16 range), where
the row order is a HOST-CHOSEN PERMUTATION of v. The permutation puts every
scatter-touched v (~2000 distinct rows) into a dedicated "hot" stripe: staging
row p*256 + c with c < 17 (slot (p,c), p=partition, 128*17 = 2176 slots). The
base stream processes staging rows as 128 partitions x 256 rows in c-chunks
[17, 48, 48, 48, 48, 47]; chunk 0 IS the hot stripe, so the entire scatter_add
reduces to one dense DVE add of the merged contribution tile into chunk 0's
SBUF tile before its store. No GPSIMD scatter, no RMW, no index tables on
device. All floating-point arithmetic runs on device; the host only reorders /
relabels / casts (permutation, fp16 cast with exact x4096 scale, slot packing).

Contribution payloads: slot (p,c) holds the attn vector (T values) of one
distinct touched v; its coefficient c4[t,a]*4096 is applied on-device via a
tiny PE matmul (block-diag c4 against a host one-hot of each slot's agent).
Duplicate v's (same v hit 2-4x, different agents -- product coefficients can't
merge) get mirror slots in columns 17/18/19 at the same partition (dup groups
are pinned to c=0, partition = dup ordinal), folded with 3 DVE adds before the
hot add. Rows >= V (OOV region) get zero vocab rows so the base stream yields
coef*0; untouched-hi overflow rows [32768, 33024) rely on PJRT-donated
pre-zeroed output buffers. Output unshard gathers rows by the permutation and
divides by 4096 (exact).
"""

import numpy as np

import concourse.bacc as bacc
import concourse.bass as bass
import concourse.mybir as mybir
import concourse.tile as tile
from concourse.bass_utils import run_bass_kernel_spmd

B, T, A, S = 8, 64, 4, 512
V, EXT_V = 32000, 33000
P = 128
KC = A * S  # 2048 contributions per batch element

SPP = 256  # staging rows per partition
NSTREAM = P * SPP  # 32768 staging rows
NSTAGE = 33024  # + 256 overflow rows (untouched OOV only)
HOTC = 17  # hot columns per partition (2176 slots >= ~2000 touched rows)
NCOL = 20  # 17 hot + 3 duplicate-mirror columns
CSTREAM = 251  # streamed columns; c in [251,256) holds untouched OOV (zeros)
ROW_CHUNKS = [HOTC, 47, 47, 47, 47, 46]  # c-chunks (sum = CSTREAM); 0 = hot
SCALE = 4096.0  # fp16 range scaling (exact power of 2)

_prog = None


class _nullctx:
    def __enter__(self):
        return None

    def __exit__(self, *a):
        return False


def _build_program(loop_n=None, ablate=()):
    """loop_n: on-device repeat loop (bench variant; outputs then meaningless).
    ablate: subset of {"items", "hotadd", "mult", "store"} (bench variants)."""
    ablate = set(ablate)
    nc = bacc.Bacc("TRN2", target_bir_lowering=False)
    f32 = mybir.dt.float32
    f16 = mybir.dt.float16
    f8 = mybir.dt.float8e4
    vocab_st = nc.dram_tensor("vocab_st", [NSTREAM, T], f8, kind="ExternalInput")
    agat_t = nc.dram_tensor("agat_t", [A, T], f32, kind="ExternalInput")
    gen_t = nc.dram_tensor("gen_t", [A, T], f32, kind="ExternalInput")
    attn_t = nc.dram_tensor("attn_t", [P, NCOL * T], f16, kind="ExternalInput")
    onehot_t = nc.dram_tensor("onehot_t", [16, 5 * P], f16, kind="ExternalInput")
    mask_t = nc.dram_tensor("mask_t", [16, 4 * T], f16, kind="ExternalInput")
    rep4_t = nc.dram_tensor("rep4_t", [A, 16], f32, kind="ExternalInput")
    out_st = nc.dram_tensor("out_st", [NSTAGE, T], f16, kind="ExternalOutput")

    do_items = "items" not in ablate
    do_hotadd = do_items and "hotadd" not in ablate
    do_mult = "mult" not in ablate
    do_store = "store" not in ablate

    with tile.TileContext(nc) as tc:
        with (
            tc.tile_pool(name="small", bufs=1) as small,
            tc.tile_pool(name="vt", bufs=3) as vtp,
            tc.tile_pool(name="sc", bufs=3) as scp,
            tc.tile_pool(name="psum1", bufs=1, space="PSUM") as psum1,
            tc.tile_pool(name="psumc", bufs=2, space="PSUM") as psumc,
            (tc.For_i(0, loop_n, 1) if loop_n else _nullctx()),
        ):
            # ---- small loads (ahead of vocab in the qSP FIFO) ----
            agat_sb = small.tile([A, T], f32)
            gen_sb = small.tile([A, T], f32)
            nc.sync.dma_start(agat_sb[:], agat_t[:])
            nc.sync.dma_start(gen_sb[:], gen_t[:])
            attn_sb = small.tile([P, NCOL * T], f16)
            nc.sync.dma_start(attn_sb[:], attn_t[:])
            onehot_sb = small.tile([16, 5 * P], f16)
            nc.sync.dma_start(onehot_sb[:], onehot_t[:])
            mask_sb = small.tile([16, 4 * T], f16)
            nc.sync.dma_start(mask_sb[:], mask_t[:])
            rep4_sb = small.tile([A, 16], f32)
            nc.sync.dma_start(rep4_sb[:], rep4_t[:])

            # ---- coefficients ----
            prod = small.tile([A, T], f32)
            nc.vector.tensor_mul(prod[:], agat_sb[:], gen_sb[:])
            ones4 = small.tile([A, P], f32)
            nc.vector.memset(ones4[:], 1.0)
            coef_ps = psum1.tile([P, T], f32, space="PSUM")
            nc.tensor.matmul(coef_ps[:], lhsT=ones4[:], rhs=prod[:], start=True, stop=True)
            coef16 = small.tile([P, T], f16)  # coef[t] on all partitions
            nc.vector.tensor_copy(coef16[:], coef_ps[:])

            c4t = small.tile([A, T], f32)  # c4T[a, t] = agent_attn*(1-gen)
            nc.vector.tensor_sub(c4t[:], agat_sb[:], prod[:])

            # rhs for per-slot coefficients: 4-chunk block-diag of c4t*SCALE.
            # rep_ps[c, t] = c4t[c%4, t] on 16 partitions (PE), then the host
            # mask (SCALE on diagonal blocks, 0 off) selects the block-diag.
            rep_ps = psum1.tile([16, T], f32, space="PSUM", tag="rep")
            nc.tensor.matmul(rep_ps[:], lhsT=rep4_sb[:], rhs=c4t[:], start=True, stop=True)
            rhs16 = small.tile([16, 4 * T], f16)
            nc.vector.tensor_tensor(
                out=rhs16[:].rearrange("p (j t) -> p j t", j=4),
                in0=mask_sb[:].rearrange("p (j t) -> p j t", j=4),
                in1=rep_ps[:, None, :].to_broadcast([16, 4, T]),
                op=mybir.AluOpType.mult,
            )

            # ---- contribution payloads: items[p, c*T+t] = attn * c4[t, a(p,c)] ----
            items = small.tile([P, NCOL * T], f16)
            if do_items:
                for g in range(5):  # 5 groups of 4 columns
                    cm = psumc.tile([P, 4 * T], f32, space="PSUM", tag="cmul")
                    nc.tensor.matmul(
                        cm[:],
                        lhsT=onehot_sb[:, g * P : (g + 1) * P],
                        rhs=rhs16[:],
                        start=True,
                        stop=True,
                    )
                    nc.vector.tensor_tensor(
                        out=items[:, g * 4 * T : (g + 1) * 4 * T],
                        in0=attn_sb[:, g * 4 * T : (g + 1) * 4 * T],
                        in1=cm[:],
                        op=mybir.AluOpType.mult,
                    )
                # fold duplicate mirrors (columns 17,18,19) into column 0
                for mc in (HOTC, HOTC + 1, HOTC + 2):
                    nc.vector.tensor_add(
                        out=items[:, 0:T],
                        in0=items[:, 0:T],
                        in1=items[:, mc * T : (mc + 1) * T],
                    )

            # coef replicated along free dim once -> chunk multiplies are flat
            # 2D step-1 fp16 ops (DVE 2x_1p mode)
            crep = max(ROW_CHUNKS)
            coefrep = small.tile([P, crep * T], f16)
            nc.vector.tensor_copy(
                out=coefrep[:].rearrange("p (c t) -> p c t", c=crep),
                in_=coef16[:, None, :].to_broadcast([P, crep, T]),
            )

            # ---- base stream: out[row, t] = coef[t] * vocab_st[row, t] ----
            vview = vocab_st[:].rearrange("(p c) t -> p c t", p=P, c=SPP)
            oview = out_st[0:NSTREAM, :].rearrange("(p c) t -> p c t", p=P, c=SPP)
            r0 = 0
            for ci, rj in enumerate(ROW_CHUNKS):
                # SWDGE load casts fp8 (HBM) -> fp16 (SBUF) inline, keeping
                # the DVE multiply in its fast 16-bit mode
                vt = vtp.tile([P, rj * T], f16, tag="vt")
                nc.gpsimd.dma_start(vt[:], vview[:, r0 : r0 + rj, :])
                src = vt
                if do_mult:
                    sc = scp.tile([P, rj * T], f16, tag="sc")
                    nc.vector.tensor_tensor(
                        out=sc[:],
                        in0=vt[:],
                        in1=coefrep[:, 0 : rj * T],
                        op=mybir.AluOpType.mult,
                    )
                    if ci == 0 and do_hotadd:
                        nc.vector.tensor_add(
                            out=sc[:], in0=sc[:], in1=items[:, 0 : HOTC * T]
                        )
                    src = sc
                if do_store:
                    nc.scalar.dma_start(oview[:, r0 : r0 + rj, :], src[:])
                r0 += rj

    nc.compile()
    return nc


def _pack_core(vocab_b, gen_b, agat_b, attn_b, article_b):
    """Host-side data layout for one batch element.

    Returns (in_map, stag_row_of_v[33000]) -- all float work is relabeling,
    an exact x4096 scale, and fp16 casts; sums/products happen on device.
    """
    v = np.asarray(article_b).reshape(-1).astype(np.int64)  # k = a*S + s
    a_of = (np.arange(KC) // S).astype(np.int64)
    attn_flat = np.ascontiguousarray(
        np.asarray(attn_b).reshape(T, KC), dtype=np.float32
    )

    vals, inv, counts = np.unique(v, return_inverse=True, return_counts=True)
    G = len(vals)
    assert G <= HOTC * P, f"touched rows {G} exceed hot capacity"
    assert counts.max() <= 4, "row multiplicity > 4 unsupported"
    dup_mask = counts >= 2
    ndup = int(dup_mask.sum())
    assert ndup <= P, f"duplicate groups {ndup} exceed {P}"

    # slot ids: dup groups first (slots 0..ndup-1 -> (p=slot, c=0)), singles after
    slot_of_group = np.empty(G, np.int64)
    slot_of_group[dup_mask] = np.arange(ndup)
    slot_of_group[~dup_mask] = ndup + np.arange(G - ndup)

    # rank of each contribution within its group (stable order)
    order = np.argsort(inv, kind="stable")
    starts = np.concatenate([[0], np.cumsum(counts)])
    rank = np.empty(KC, np.int64)
    rank[order] = np.arange(KC) - starts[inv[order]]

    slot_k = slot_of_group[inv]
    part_k = slot_k % P
    col_k = np.where(rank == 0, slot_k // P, HOTC - 1 + rank)

    attnT = np.zeros((P, NCOL, T), np.float32)
    attnT[part_k, col_k, :] = attn_flat.T  # [KC, T] -> slots
    onehot = np.zeros((16, 5, P), np.float32)
    onehot[(col_k % 4) * A + a_of, col_k // 4, part_k] = 1.0

    # permutation: touched v -> hot rows; untouched fill the rest
    stag_row_of_v = np.empty(EXT_V, np.int64)
    rows_touched = (slot_of_group % P) * SPP + slot_of_group // P
    stag_row_of_v[vals] = rows_touched
    free_stream = np.setdiff1d(np.arange(NSTREAM), rows_touched, assume_unique=False)
    touched_mask = np.zeros(EXT_V, bool)
    touched_mask[vals] = True
    unt_vocab = np.nonzero(~touched_mask[:V])[0]
    unt_hi = np.nonzero(~touched_mask[V:])[0] + V
    # vocab rows must land in streamed columns (c < CSTREAM); untouched OOV
    # rows (zero data) preferentially fill never-streamed rows: the overflow
    # region and the tail columns c in [CSTREAM, SPP)
    lo_mask = (free_stream % SPP) < CSTREAM
    free_lo = free_stream[lo_mask]
    free_tail = free_stream[~lo_mask]
    assert len(unt_vocab) <= len(free_lo), "stream cannot hold vocab rows"
    stag_row_of_v[unt_vocab] = free_lo[: len(unt_vocab)]
    hi_rows = np.concatenate(
        [
            np.arange(NSTREAM, NSTAGE),
            free_tail,
            free_lo[len(unt_vocab) :],
        ]
    )
    assert len(unt_hi) <= len(hi_rows), "OOV overflow exceeded"
    stag_row_of_v[unt_hi] = hi_rows[: len(unt_hi)]

    f8np = mybir.dt.np(mybir.dt.float8e4)
    vocab_st = np.zeros((NSTREAM, T), f8np)
    vocab_st[stag_row_of_v[:V]] = (
        np.asarray(vocab_b).T.astype(np.float32) * SCALE
    ).astype(f8np)

    mask = np.zeros((16, 4, T), np.float16)
    for j in range(4):
        mask[4 * j : 4 * (j + 1), j, :] = SCALE
    rep4 = np.zeros((A, 16), np.float32)
    rep4[np.arange(16) % A, np.arange(16)] = 1.0

    in_map = {
        "vocab_st": vocab_st,
        "agat_t": np.ascontiguousarray(np.asarray(agat_b).T, dtype=np.float32),
        "gen_t": np.ascontiguousarray(np.asarray(gen_b).T, dtype=np.float32),
        "attn_t": attnT.reshape(P, NCOL * T).astype(np.float16),
        "onehot_t": onehot.reshape(16, 5 * P).astype(np.float16),
        "mask_t": mask.reshape(16, 4 * T),
        "rep4_t": rep4,
    }
    return in_map, stag_row_of_v


def kernel(vocab_probs, generation_probs, agentwise_attn, agent_attn, article):
    global _prog
    vocab_probs = np.asarray(vocab_probs, dtype=np.float32)
    generation_probs = np.asarray(generation_probs, dtype=np.float32)
    agentwise_attn = np.asarray(agentwise_attn, dtype=np.float32)
    agent_attn = np.asarray(agent_attn, dtype=np.float32)
    article = np.asarray(article)

    if _prog is None:
        _prog = _build_program()

    packed = [
        _pack_core(
            vocab_probs[b], generation_probs[b], agat_b=agent_attn[b],
            attn_b=agentwise_attn[b], article_b=article[b],
        )
        for b in range(B)
    ]
    in_maps = [p[0] for p in packed]
    res = run_bass_kernel_spmd(_prog, in_maps, core_ids=list(range(B)))
    full = np.empty((B, T, EXT_V), np.float32)
    inv_scale = np.float32(1.0 / SCALE)
    for b, r in enumerate(res.results):
        stag = np.asarray(r["out_st"])
        full[b] = stag[packed[b][1]].astype(np.float32).T * inv_scale
    return full


# revision 26
# speedup vs baseline: 6.8521x; 6.8521x over previous
"""Trainium2 Bass kernel for nn_MultiAgentsSummarizer (pointer-generator style
multi-agent summarizer distribution).

Math (per batch b, with T=64 target positions, A=4 agents, S=512 source tokens,
V=32000 vocab, EXT_V=33000 extended vocab):

    coef[t]   = sum_a agent_attn[t,a] * gen[t,a]
    out[t,v]  = coef[t] * vocab_probs[t,v]            (v <  V;  0 for v >= V)
    out[t, article[a,s]] += agent_attn[t,a]*(1-gen[t,a]) * agentwise_attn[t,a,s]

Strategy: one batch element per NeuronCore (B=8 = n_cores). Device work runs in
a v-major staging layout S[row, t] (fp16, x4096 scaling for fp16 range), where
the row order is a HOST-CHOSEN PERMUTATION of v. The permutation puts every
scatter-touched v (~2000 distinct rows) into a dedicated "hot" stripe: staging
row p*256 + c with c < 17 (slot (p,c), p=partition, 128*17 = 2176 slots). The
base stream processes staging rows as 128 partitions x 256 rows in c-chunks
ROW_CHUNKS; chunk 0 IS the hot stripe, so the entire scatter_add
reduces to one dense DVE add of the merged contribution tile into chunk 0's
SBUF tile before its store. No GPSIMD scatter, no RMW, no index tables on
device. All floating-point arithmetic runs on device; the host only reorders /
relabels / casts (permutation, fp16 cast with exact x4096 scale, slot packing).

Contribution payloads: slot (p,c) holds the attn vector (T values) of one
distinct touched v; its coefficient c4[t,a]*4096 is applied on-device via a
tiny PE matmul (block-diag c4 against a host one-hot of each slot's agent).
Duplicate v's (same v hit 2-4x, different agents -- product coefficients can't
merge) get mirror slots in columns 17/18/19 at the same partition (dup groups
are pinned to c=0, partition = dup ordinal), folded with 3 DVE adds before the
hot add. Rows >= V (OOV region) get zero vocab rows so the base stream yields
coef*0; untouched-hi overflow rows [32768, 33024) rely on PJRT-donated
pre-zeroed output buffers. Output unshard gathers rows by the permutation and
divides by 4096 (exact).
"""

import numpy as np

import concourse.bacc as bacc
import concourse.bass as bass
import concourse.mybir as mybir
import concourse.tile as tile
from concourse.bass_utils import run_bass_kernel_spmd

B, T, A, S = 8, 64, 4, 512
V, EXT_V = 32000, 33000
P = 128
KC = A * S  # 2048 contributions per batch element

SPP = 256  # staging rows per partition
NSTREAM = P * SPP  # 32768 staging rows
NSTAGE = 33024  # + 256 overflow rows (untouched OOV only)
HOTC = 17  # hot columns per partition (2176 slots >= ~2000 touched rows)
NCOL = 20  # 17 hot + 3 duplicate-mirror columns
CSTREAM = 251  # streamed columns; c in [251,256) holds untouched OOV (zeros)
ROW_CHUNKS = [HOTC, 47, 47, 47, 47, 46]  # c-chunks (sum = CSTREAM); 0 = hot
SCALE = 4096.0  # fp16 range scaling (exact power of 2)

_prog = None

# vocab load path: "f16" (fp16 HWDGE load), "f8_dve" (fp8 load, DVE reads fp8),
# "f8_act" (fp8 load, ACT-engine cast to fp16, DVE multiplies fp16)
VOCAB_MODE = "f16"


class _nullctx:
    def __enter__(self):
        return None

    def __exit__(self, *a):
        return False


def _build_program(loop_n=None, ablate=(), vocab_mode=None, chunks=None):
    """loop_n: on-device repeat loop (bench variant; outputs then meaningless).
    ablate: subset of {"items", "hotadd", "mult", "store"} (bench variants)."""
    ablate = set(ablate)
    vocab_mode = vocab_mode or VOCAB_MODE
    row_chunks = chunks or ROW_CHUNKS
    assert sum(row_chunks) == CSTREAM and row_chunks[0] == HOTC
    nc = bacc.Bacc("TRN2", target_bir_lowering=False)
    f32 = mybir.dt.float32
    f16 = mybir.dt.float16
    f8 = mybir.dt.float8e4
    vdt = f16 if vocab_mode == "f16" else f8
    vocab_st = nc.dram_tensor("vocab_st", [NSTREAM, T], vdt, kind="ExternalInput")
    agat_t = nc.dram_tensor("agat_t", [A, T], f32, kind="ExternalInput")
    gen_t = nc.dram_tensor("gen_t", [A, T], f32, kind="ExternalInput")
    attn_t = nc.dram_tensor("attn_t", [P, NCOL * T], f16, kind="ExternalInput")
    onehot_t = nc.dram_tensor("onehot_t", [16, 5 * P], f16, kind="ExternalInput")
    mask_t = nc.dram_tensor("mask_t", [16, 4 * T], f16, kind="ExternalInput")
    rep4_t = nc.dram_tensor("rep4_t", [A, 16], f32, kind="ExternalInput")
    out_st = nc.dram_tensor("out_st", [NSTAGE, T], f16, kind="ExternalOutput")

    do_items = "items" not in ablate
    do_hotadd = do_items and "hotadd" not in ablate
    do_mult = "mult" not in ablate
    do_store = "store" not in ablate

    with tile.TileContext(nc) as tc:
        with (
            tc.tile_pool(name="small", bufs=1) as small,
            tc.tile_pool(name="vt", bufs=3) as vtp,
            tc.tile_pool(name="v16", bufs=3) as v16p,
            tc.tile_pool(name="sc", bufs=3) as scp,
            tc.tile_pool(name="psum1", bufs=1, space="PSUM") as psum1,
            tc.tile_pool(name="psumc", bufs=2, space="PSUM") as psumc,
            (tc.For_i(0, loop_n, 1) if loop_n else _nullctx()),
        ):
            # ---- small loads (ahead of vocab in the qSP FIFO) ----
            agat_sb = small.tile([A, T], f32)
            gen_sb = small.tile([A, T], f32)
            nc.sync.dma_start(agat_sb[:], agat_t[:])
            nc.sync.dma_start(gen_sb[:], gen_t[:])
            attn_sb = small.tile([P, NCOL * T], f16)
            nc.sync.dma_start(attn_sb[:], attn_t[:])
            onehot_sb = small.tile([16, 5 * P], f16)
            nc.sync.dma_start(onehot_sb[:], onehot_t[:])
            mask_sb = small.tile([16, 4 * T], f16)
            nc.sync.dma_start(mask_sb[:], mask_t[:])
            rep4_sb = small.tile([A, 16], f32)
            nc.sync.dma_start(rep4_sb[:], rep4_t[:])

            # ---- coefficients ----
            prod = small.tile([A, T], f32)
            nc.vector.tensor_mul(prod[:], agat_sb[:], gen_sb[:])
            ones4 = small.tile([A, P], f32)
            nc.vector.memset(ones4[:], 1.0)
            coef_ps = psum1.tile([P, T], f32, space="PSUM")
            nc.tensor.matmul(coef_ps[:], lhsT=ones4[:], rhs=prod[:], start=True, stop=True)
            coef16 = small.tile([P, T], f16)  # coef[t] on all partitions
            nc.vector.tensor_copy(coef16[:], coef_ps[:])

            c4t = small.tile([A, T], f32)  # c4T[a, t] = agent_attn*(1-gen)
            nc.vector.tensor_sub(c4t[:], agat_sb[:], prod[:])

            # rhs for per-slot coefficients: 4-chunk block-diag of c4t*SCALE.
            # rep_ps[c, t] = c4t[c%4, t] on 16 partitions (PE), then the host
            # mask (SCALE on diagonal blocks, 0 off) selects the block-diag.
            rep_ps = psum1.tile([16, T], f32, space="PSUM", tag="rep")
            nc.tensor.matmul(rep_ps[:], lhsT=rep4_sb[:], rhs=c4t[:], start=True, stop=True)
            rhs16 = small.tile([16, 4 * T], f16)
            nc.vector.tensor_tensor(
                out=rhs16[:].rearrange("p (j t) -> p j t", j=4),
                in0=mask_sb[:].rearrange("p (j t) -> p j t", j=4),
                in1=rep_ps[:, None, :].to_broadcast([16, 4, T]),
                op=mybir.AluOpType.mult,
            )

            # ---- contribution payloads: items[p, c*T+t] = attn * c4[t, a(p,c)] ----
            items = small.tile([P, NCOL * T], f16)
            if do_items:
                for g in range(5):  # 5 groups of 4 columns
                    cm = psumc.tile([P, 4 * T], f32, space="PSUM", tag="cmul")
                    nc.tensor.matmul(
                        cm[:],
                        lhsT=onehot_sb[:, g * P : (g + 1) * P],
                        rhs=rhs16[:],
                        start=True,
                        stop=True,
                    )
                    nc.vector.tensor_tensor(
                        out=items[:, g * 4 * T : (g + 1) * 4 * T],
                        in0=attn_sb[:, g * 4 * T : (g + 1) * 4 * T],
                        in1=cm[:],
                        op=mybir.AluOpType.mult,
                    )
                # fold duplicate mirrors (columns 17,18,19) into column 0
                for mc in (HOTC, HOTC + 1, HOTC + 2):
                    nc.vector.tensor_add(
                        out=items[:, 0:T],
                        in0=items[:, 0:T],
                        in1=items[:, mc * T : (mc + 1) * T],
                    )

            # coef replicated along free dim once -> chunk multiplies are flat
            # 2D step-1 fp16 ops (DVE 2x_1p mode)
            crep = max(row_chunks)
            coefrep = small.tile([P, crep * T], f16)
            nc.vector.tensor_copy(
                out=coefrep[:].rearrange("p (c t) -> p c t", c=crep),
                in_=coef16[:, None, :].to_broadcast([P, crep, T]),
            )

            # ---- base stream: out[row, t] = coef[t] * vocab_st[row, t] ----
            vview = vocab_st[:].rearrange("(p c) t -> p c t", p=P, c=SPP)
            oview = out_st[0:NSTREAM, :].rearrange("(p c) t -> p c t", p=P, c=SPP)
            r0 = 0
            for ci, rj in enumerate(row_chunks):
                vt = vtp.tile([P, rj * T], vdt, tag="vt")
                ld_eng = nc.scalar if ("splitload" in ablate and ci % 2) else nc.sync
                ld_eng.dma_start(vt[:], vview[:, r0 : r0 + rj, :])
                src = vt
                if do_mult:
                    if vocab_mode == "f8_act":
                        vt16 = v16p.tile([P, rj * T], f16, tag="vt16")
                        nc.scalar.activation(
                            vt16[:], vt[:], mybir.ActivationFunctionType.Copy
                        )
                        vt = vt16
                    sc = scp.tile([P, rj * T], f16, tag="sc")
                    nc.vector.tensor_tensor(
                        out=sc[:],
                        in0=vt[:],
                        in1=coefrep[:, 0 : rj * T],
                        op=mybir.AluOpType.mult,
                    )
                    if ci == 0 and do_hotadd:
                        nc.vector.tensor_add(
                            out=sc[:], in0=sc[:], in1=items[:, 0 : HOTC * T]
                        )
                    src = sc
                if do_store:
                    nc.scalar.dma_start(oview[:, r0 : r0 + rj, :], src[:])
                r0 += rj

    nc.compile()
    return nc


def _pack_core(vocab_b, gen_b, agat_b, attn_b, article_b, vocab_mode=None):
    """Host-side data layout for one batch element.

    Returns (in_map, stag_row_of_v[33000]) -- all float work is relabeling,
    an exact x4096 scale, and fp16 casts; sums/products happen on device.
    """
    v = np.asarray(article_b).reshape(-1).astype(np.int64)  # k = a*S + s
    a_of = (np.arange(KC) // S).astype(np.int64)
    attn_flat = np.ascontiguousarray(
        np.asarray(attn_b).reshape(T, KC), dtype=np.float32
    )

    vals, inv, counts = np.unique(v, return_inverse=True, return_counts=True)
    G = len(vals)
    assert G <= HOTC * P, f"touched rows {G} exceed hot capacity"
    assert counts.max() <= 4, "row multiplicity > 4 unsupported"
    dup_mask = counts >= 2
    ndup = int(dup_mask.sum())
    assert ndup <= P, f"duplicate groups {ndup} exceed {P}"

    # slot ids: dup groups first (slots 0..ndup-1 -> (p=slot, c=0)), singles after
    slot_of_group = np.empty(G, np.int64)
    slot_of_group[dup_mask] = np.arange(ndup)
    slot_of_group[~dup_mask] = ndup + np.arange(G - ndup)

    # rank of each contribution within its group (stable order)
    order = np.argsort(inv, kind="stable")
    starts = np.concatenate([[0], np.cumsum(counts)])
    rank = np.empty(KC, np.int64)
    rank[order] = np.arange(KC) - starts[inv[order]]

    slot_k = slot_of_group[inv]
    part_k = slot_k % P
    col_k = np.where(rank == 0, slot_k // P, HOTC - 1 + rank)

    attnT = np.zeros((P, NCOL, T), np.float32)
    attnT[part_k, col_k, :] = attn_flat.T  # [KC, T] -> slots
    onehot = np.zeros((16, 5, P), np.float32)
    onehot[(col_k % 4) * A + a_of, col_k // 4, part_k] = 1.0

    # permutation: touched v -> hot rows; untouched fill the rest
    stag_row_of_v = np.empty(EXT_V, np.int64)
    rows_touched = (slot_of_group % P) * SPP + slot_of_group // P
    stag_row_of_v[vals] = rows_touched
    free_stream = np.setdiff1d(np.arange(NSTREAM), rows_touched, assume_unique=False)
    touched_mask = np.zeros(EXT_V, bool)
    touched_mask[vals] = True
    unt_vocab = np.nonzero(~touched_mask[:V])[0]
    unt_hi = np.nonzero(~touched_mask[V:])[0] + V
    # vocab rows must land in streamed columns (c < CSTREAM); untouched OOV
    # rows (zero data) preferentially fill never-streamed rows: the overflow
    # region and the tail columns c in [CSTREAM, SPP)
    lo_mask = (free_stream % SPP) < CSTREAM
    free_lo = free_stream[lo_mask]
    free_tail = free_stream[~lo_mask]
    assert len(unt_vocab) <= len(free_lo), "stream cannot hold vocab rows"
    stag_row_of_v[unt_vocab] = free_lo[: len(unt_vocab)]
    hi_rows = np.concatenate(
        [
            np.arange(NSTREAM, NSTAGE),
            free_tail,
            free_lo[len(unt_vocab) :],
        ]
    )
    assert len(unt_hi) <= len(hi_rows), "OOV overflow exceeded"
    stag_row_of_v[unt_hi] = hi_rows[: len(unt_hi)]

    vocab_mode = vocab_mode or VOCAB_MODE
    vnp = np.float16 if vocab_mode == "f16" else mybir.dt.np(mybir.dt.float8e4)
    vocab_st = np.zeros((NSTREAM, T), vnp)
    vocab_st[stag_row_of_v[:V]] = (
        np.asarray(vocab_b).T.astype(np.float32) * SCALE
    ).astype(vnp)

    mask = np.zeros((16, 4, T), np.float16)
    for j in range(4):
        mask[4 * j : 4 * (j + 1), j, :] = SCALE
    rep4 = np.zeros((A, 16), np.float32)
    rep4[np.arange(16) % A, np.arange(16)] = 1.0

    in_map = {
        "vocab_st": vocab_st,
        "agat_t": np.ascontiguousarray(np.asarray(agat_b).T, dtype=np.float32),
        "gen_t": np.ascontiguousarray(np.asarray(gen_b).T, dtype=np.float32),
        "attn_t": attnT.reshape(P, NCOL * T).astype(np.float16),
        "onehot_t": onehot.reshape(16, 5 * P).astype(np.float16),
        "mask_t": mask.reshape(16, 4 * T),
        "rep4_t": rep4,
    }
    return in_map, stag_row_of_v


def kernel(vocab_probs, generation_probs, agentwise_attn, agent_attn, article):
    global _prog
    vocab_probs = np.asarray(vocab_probs, dtype=np.float32)
    generation_probs = np.asarray(generation_probs, dtype=np.float32)
    agentwise_attn = np.asarray(agentwise_attn, dtype=np.float32)
    agent_attn = np.asarray(agent_attn, dtype=np.float32)
    article = np.asarray(article)

    if _prog is None:
        _prog = _build_program()

    packed = [
        _pack_core(
            vocab_probs[b], generation_probs[b], agat_b=agent_attn[b],
            attn_b=agentwise_attn[b], article_b=article[b],
        )
        for b in range(B)
    ]
    in_maps = [p[0] for p in packed]
    res = run_bass_kernel_spmd(_prog, in_maps, core_ids=list(range(B)))
    full = np.empty((B, T, EXT_V), np.float32)
    inv_scale = np.float32(1.0 / SCALE)
    for b, r in enumerate(res.results):
        stag = np.asarray(r["out_st"])
        full[b] = stag[packed[b][1]].astype(np.float32).T * inv_scale
    return full
